# revision 1
# baseline (speedup 1.0000x reference)
"""Trainium2 Bass kernel for nn_CriticNetwork (gnn_message_passing).

Key mathematical simplification (verified numerically against the
reference): the reference broadcasts edge_index to (B, 2, E) and
reshapes to (2, B*E).  Row-major reshape interleaves the src/dst
blocks so the resulting src and dst arrays are ELEMENTWISE EQUAL --
every edge is a self-edge v->v.  With GCN normalization
(deg = 1 + 2*count(v), each self-edge contributes x[v]/deg, plus the
explicit self-loop) the aggregate is exactly deg * x[v]/deg = x[v].
Both GCNConv layers therefore collapse to plain linear layers:

    x = relu(x @ W1 + b1); x = relu(x @ W2 + b2)
    node_avg[b] = mean_n(x[b, n] @ node_fc_W) + node_fc_b
    col path is a plain 2-layer MLP; final head is a tiny [4,2] MLP.

Since node_fc / col_W2 are applied linearly after the last relu, the
device only needs per-(batch-slice) SUMS of the hidden activations:
each core processes 25000 nodes (half a batch) + 500 col rows and
returns two small accumulator vectors; the host applies the final
(tiny) linear head.

Device layout per core:
  xT_packed [128, 12500]: rows 0-63  = 64 features of nodes [0, 12500)
                          rows 64-127 = 64 features of nodes [12500, 25000)
  L1 matmul: lhsT = blockdiag(W1, W1) [128, 32] -> h1.T bands [32, 512]
  4 L1 matmuls stack bands in one PSUM bank -> [128, 512]
  relu (ScalarE, bias fused) -> SBUF
  L2 matmul: lhsT = blockdiag(W2 x8) [128, 128] -> [128, 512] PSUM
  relu + accumulate (ScalarE accum_out = per-partition row sum)
  final: reduce accum columns -> node_acc [128, 1] (8 bands of 16)

All constants (weights, biases, col features) ship in ONE packed DMA
("wpack") and a zero-valued warmup matmul consumes it first: the PE
LDWEIGHTS instruction can carry only ONE semaphore wait, so every real
matmul must depend on at most one un-synced DMA lane (its x chunk).
"""

import ml_dtypes
import numpy as np

import concourse.bacc as bacc
import concourse.bass as bass
import concourse.mybir as mybir
import concourse.tile as tile
from concourse.bass_utils import run_bass_kernel_spmd

P = 128
N_CORES = 8
B, N, F_NODE, H = 4, 50000, 64, 16
NODES_PER_CORE = (B * N) // N_CORES        # 25000
COLS = NODES_PER_CORE // 2                 # 12500 packed columns (2 nodes/col)
MM = 512                                   # fp32 matmul max moving free dim
SUPER = 4 * MM                             # 2048 columns per PSUM-bank group
N_CHUNKS = (COLS + SUPER - 1) // SUPER     # 7 (6 full + 212-col tail)
C, F_COL = 1000, 32
COLN = (B * C) // N_CORES                  # 500 col rows per core

# wpack column layout
W1_OFF = 0                                  # [128, 32] blockdiag(W1, W1)
W2_OFF = W1_OFF + 2 * H                     # [128, 128] blockdiag(W2 x8)
B1_OFF = W2_OFF + P                         # [128, 1] b1 tiled x8
B2_OFF = B1_OFF + 1                         # [128, 1] b2 tiled x8
CW1_OFF = B2_OFF + 1                        # [32, 16] col_W1 (rows 0-31)
CB1_OFF = CW1_OFF + H                       # [16, 1] col_b1 (rows 0-15)
ZPAD_OFF = CB1_OFF + 1                      # [128, 1] zeros (warmup operand)
COLT_OFF = ZPAD_OFF + 1                     # [32, 500] colT (rows 0-31)
NW = COLT_OFF + COLN                        # 680

DT = mybir.dt.bfloat16                     # matmul-operand dtype on device
NPDT = ml_dtypes.bfloat16

PROFILE = False        # set True (e.g. from test.py) to collect NTFF timing
CHECK_WAITS = True     # build-time guard: one semaphore wait per compute inst
LAST_EXEC_TIME_NS = None
LAST_RESULTS = None

_NC_CACHE = {}


def _build_nc(relu1_on_dve=True):
    f32 = mybir.dt.float32
    Relu = mybir.ActivationFunctionType.Relu
    # Bacc (not raw Bass): its finalize() runs move_matmul_waits_to_-
    # ldweights + generate_event_semaphores, which legalize schedules
    # against the TRN2 one-semaphore-wait-per-instruction limit.
    nc = bacc.Bacc("TRN2")

    xT = nc.dram_tensor("xT", [P, COLS], DT, kind="ExternalInput")
    wpack = nc.dram_tensor("wpack", [P, NW], DT, kind="ExternalInput")
    node_acc = nc.dram_tensor("node_acc", [P, 1], f32, kind="ExternalOutput")
    col_acc = nc.dram_tensor("col_acc", [H, 1], f32, kind="ExternalOutput")

    with tile.TileContext(nc) as tc:
        with (
            tc.tile_pool(name="consts", bufs=1) as consts,
            tc.tile_pool(name="xin", bufs=4) as xin,
            tc.tile_pool(name="work", bufs=2) as work,
            tc.tile_pool(name="outp", bufs=1) as outp,
            tc.tile_pool(name="psum", bufs=1, space="PSUM") as psum,
        ):
            wp = consts.tile([P, NW], DT)
            nc.sync.dma_start(wp[:], wpack[:])
            w1_t = wp[:, W1_OFF:W1_OFF + 2 * H]
            w2_t = wp[:, W2_OFF:W2_OFF + P]
            b1_t = wp[:, B1_OFF:B1_OFF + 1]
            b2_t = wp[:, B2_OFF:B2_OFF + 1]
            cw1_t = wp[:F_COL, CW1_OFF:CW1_OFF + H]
            cb1_t = wp[:H, CB1_OFF:CB1_OFF + 1]
            zc_t = wp[:, ZPAD_OFF:ZPAD_OFF + 1]
            colT_t = wp[:F_COL, COLT_OFF:COLT_OFF + COLN]

            # Zero stats ON the engine that will accumulate into it (same-
            # engine WAW needs no cross-engine wait).  Reading wpack here
            # also syncs that engine with the wpack DMA lane up front.
            # zeros path: everything post-PE lives on DVE and the Scalar
            # engine is left completely idle (no ACT_TABLE_LOAD either).
            stats = outp.tile([P, N_CHUNKS + 1], f32)
            if relu1_on_dve:
                nc.vector.tensor_scalar_mul(stats[:], wp[:, :N_CHUNKS + 1], 0.0)
            else:
                nc.scalar.mul(stats[:], wp[:, :N_CHUNKS + 1], 0.0)

            # Persistent PSUM tiles (allocated once, manually alternated):
            # a per-chunk pool tile would get a slot-recycle writer guard,
            # an extra PE-sem wait on the first matmul of each chunk -- and
            # the PE LDWEIGHTS instruction can carry only ONE wait.
            NBUF = 3
            ps1_t = [psum.tile([P, MM], f32, tag=f"ps1_{k}", name=f"ps1_{k}")
                     for k in range(NBUF)]
            ps2_t = [psum.tile([P, MM], f32, tag=f"ps2_{k}", name=f"ps2_{k}")
                     for k in range(NBUF)]
            h1r_t = [work.tile([P, MM], DT, tag=f"h1r_{k}", name=f"h1r_{k}")
                     for k in range(NBUF)]
            scr_t = [work.tile([P, MM], DT, tag=f"scr_{k}", name=f"scr_{k}")
                     for k in range(NBUF)]

            # Warmup matmul: syncs PE with the wpack DMA using a single
            # wait, so every later matmul has the wpack lane subsumed.
            # Reads the zero pad column -> contributes exactly 0.0 to
            # stats' spare column (kept live through that write).
            psd = psum.tile([1, 1], f32, tag="psd")
            nc.tensor.matmul(psd[0:1, 0:1], zc_t, zc_t, start=True, stop=True)
            if relu1_on_dve:
                nc.vector.tensor_copy(stats[0:1, N_CHUNKS:N_CHUNKS + 1],
                                      psd[0:1, 0:1])
            else:
                nc.scalar.copy(stats[0:1, N_CHUNKS:N_CHUNKS + 1], psd[0:1, 0:1])

            for s in range(N_CHUNKS):
                c0 = s * SUPER
                cols = min(SUPER, COLS - c0)
                nb = (cols + MM - 1) // MM
                act_w = cols if nb == 1 else cols // nb
                assert act_w * nb == cols, (s, cols, nb)

                x_t = xin.tile([P, SUPER], DT, tag="x")
                nc.sync.dma_start(x_t[:, :cols], xT[:, c0:c0 + cols])

                ps1 = ps1_t[s % NBUF]
                for bnd in range(nb):
                    w = min(MM, cols - bnd * MM)
                    nc.tensor.matmul(
                        ps1[32 * bnd:32 * bnd + 32, :w],
                        w1_t,
                        x_t[:, bnd * MM:bnd * MM + w],
                        start=True, stop=True,
                        tile_position=(0, 32 * bnd),
                    )
                used = 32 * nb

                h1r = h1r_t[s % NBUF]
                if relu1_on_dve:
                    # b1 is structurally zero (setup_inputs uses
                    # jnp.zeros), so relu1 is a plain max with an
                    # immediate -- keeps DVE free of a wpack-DMA wait.
                    nc.vector.tensor_scalar_max(
                        h1r[:used, :act_w], ps1[:used, :act_w], 0.0)
                else:
                    nc.scalar.activation(
                        h1r[:used, :act_w], ps1[:used, :act_w], Relu,
                        bias=b1_t[:used, :],
                    )

                ps2 = ps2_t[s % NBUF]
                nc.tensor.matmul(
                    ps2[:used, :act_w],
                    w2_t[:used, :used],
                    h1r[:used, :act_w],
                    start=True, stop=True,
                )
                scr = scr_t[s % NBUF]
                if relu1_on_dve:
                    # b2 structurally zero: relu2 + row-sum in one DVE op.
                    nc.vector.tensor_scalar(
                        scr[:used, :act_w], ps2[:used, :act_w], 0.0, 0.0,
                        mybir.AluOpType.max, mybir.AluOpType.add,
                        accum_out=stats[:used, s:s + 1],
                    )
                else:
                    nc.scalar.activation(
                        scr[:used, :act_w], ps2[:used, :act_w], Relu,
                        bias=b2_t[:used, :],
                        accum_out=stats[:used, s:s + 1],
                    )

            # column-features path (tiny): h = relu(col @ col_W1 + col_b1)
            psc = psum.tile([H, COLN], f32, tag="psc")
            nc.tensor.matmul(psc[:, :], cw1_t, colT_t, start=True, stop=True)
            colscr = outp.tile([H, COLN], f32)
            col_sb = outp.tile([H, 1], f32)
            if relu1_on_dve:
                # col_b1 structurally zero as well.
                nc.vector.tensor_scalar(
                    colscr[:], psc[:], 0.0, 0.0,
                    mybir.AluOpType.max, mybir.AluOpType.add,
                    accum_out=col_sb[:])
            else:
                nc.scalar.activation(colscr[:], psc[:], Relu,
                                     bias=cb1_t, accum_out=col_sb[:])

            node_sb = outp.tile([P, 1], f32)
            nc.vector.tensor_reduce(node_sb[:], stats[:],
                                    axis=mybir.AxisListType.X,
                                    op=mybir.AluOpType.add)
            nc.sync.dma_start(node_acc[:], node_sb[:])
            nc.sync.dma_start(col_acc[:], col_sb[:])

    nc.finalize()

    # Verify the legalization: at most one wait per instruction
    # (InstEventSemaphore may carry two).
    if CHECK_WAITS:
        for blk in nc.m.functions[0].blocks:
            for inst in blk.instructions:
                si = inst.sync_info
                nwait = len(si.on_wait) if si and si.on_wait else 0
                limit = 2 if type(inst).__name__ in (
                    "InstEventSemaphore", "InstDrain", "InstDMACopy") else 1
                assert nwait <= limit, (
                    inst.name, type(inst).__name__,
                    [w.ant_name for w in si.on_wait])
    return nc


def _get_nc(relu1_on_dve=True):
    key = ("nc", relu1_on_dve)
    if key not in _NC_CACHE:
        _NC_CACHE[key] = _build_nc(relu1_on_dve)
    return _NC_CACHE[key]


def _prep_in_maps(node_features, col_features, W1, b1, W2, b2, col_W1, col_b1):
    x = np.ascontiguousarray(node_features, dtype=np.float32).reshape(B * N, F_NODE)
    colf = np.ascontiguousarray(col_features, dtype=np.float32).reshape(B * C, F_COL)

    W1 = np.asarray(W1, np.float32)
    W2 = np.asarray(W2, np.float32)
    wpack = np.zeros((P, NW), np.float32)
    wpack[:F_NODE, W1_OFF:W1_OFF + H] = W1
    wpack[F_NODE:, W1_OFF + H:W1_OFF + 2 * H] = W1
    for i in range(P // H):
        wpack[H * i:H * i + H, W2_OFF + H * i:W2_OFF + H * i + H] = W2
    wpack[:, B1_OFF] = np.tile(np.asarray(b1, np.float32), P // H)
    wpack[:, B2_OFF] = np.tile(np.asarray(b2, np.float32), P // H)
    wpack[:F_COL, CW1_OFF:CW1_OFF + H] = np.asarray(col_W1, np.float32)
    wpack[:H, CB1_OFF] = np.asarray(col_b1, np.float32)

    in_maps = []
    for c in range(N_CORES):
        n0 = c * NODES_PER_CORE
        half = NODES_PER_CORE // 2
        xa = x[n0:n0 + half].T                      # [64, 12500] view
        xb = x[n0 + half:n0 + NODES_PER_CORE].T
        xT = np.ascontiguousarray(
            np.concatenate([xa, xb], axis=0), dtype=np.float32).astype(NPDT)
        wp = wpack.copy()
        wp[:F_COL, COLT_OFF:COLT_OFF + COLN] = colf[c * COLN:(c + 1) * COLN].T
        in_maps.append({"xT": xT, "wpack": wp.astype(NPDT)})
    return in_maps


def kernel(node_features, col_features, edge_index, W1, b1, W2, b2,
           node_fc_W, node_fc_b, col_W1, col_b1, col_W2, col_b2,
           fc_W, fc_b, out_W, out_b):
    global LAST_EXEC_TIME_NS, LAST_RESULTS
    # edge_index provably does not affect the output (see module docstring).
    in_maps = _prep_in_maps(node_features, col_features,
                            W1, b1, W2, b2, col_W1, col_b1)
    zeros_path = not (np.any(np.asarray(b1)) or np.any(np.asarray(b2))
                      or np.any(np.asarray(col_b1)))
    nc = _get_nc(relu1_on_dve=zeros_path)
    res = run_bass_kernel_spmd(nc, in_maps, core_ids=list(range(N_CORES)),
                               trace=PROFILE)
    LAST_EXEC_TIME_NS = res.exec_time_ns
    LAST_RESULTS = res
    outs = res.results

    node_fc_W = np.asarray(node_fc_W, np.float32)
    col_W2 = np.asarray(col_W2, np.float32)
    node_avg = np.zeros((B, 1), np.float32)
    col_avg = np.zeros((B, 1), np.float32)
    for b in range(B):
        ns = (outs[2 * b]["node_acc"].reshape(P // H, H).sum(axis=0) +
              outs[2 * b + 1]["node_acc"].reshape(P // H, H).sum(axis=0))
        cs = (outs[2 * b]["col_acc"].reshape(H) +
              outs[2 * b + 1]["col_acc"].reshape(H))
        node_avg[b, 0] = (ns / np.float32(N)) @ node_fc_W[:, 0] + \
            np.asarray(node_fc_b, np.float32)[0]
        col_avg[b, 0] = (cs / np.float32(C)) @ col_W2[:, 0] + \
            np.asarray(col_b2, np.float32)[0]

    combined = np.concatenate([node_avg, col_avg], axis=1)      # [B, 2]
    z = np.maximum(combined @ np.asarray(fc_W, np.float32) +
                   np.asarray(fc_b, np.float32), 0.0)
    out = z @ np.asarray(out_W, np.float32) + np.asarray(out_b, np.float32)
    return out.astype(np.float32)



# revision 7
# speedup vs baseline: 1.2825x; 1.2825x over previous
"""Trainium2 Bass kernel for nn_CriticNetwork (gnn_message_passing).

Math (verified vs reference): the reference broadcasts edge_index to
(B, 2, E) and reshapes to (2, B*E); row-major interleaving makes src and
dst elementwise equal, so every edge is a self-edge and both GCNConv
layers collapse to plain linear layers (deg*x/deg = x):

    x = relu(x @ W1); x = relu(x @ W2)        (biases are zeros)
    node_avg[b] = mean_n(x[b,n] @ node_fc_W) + node_fc_b
    col path is a tiny 2-layer MLP; final head is a [4,2] MLP.

Device strategy (per core, 25000 nodes = half a batch):
  * node path runs in fp8e4m3 end to end: x, W1, W2 and the hidden
    activations are fp8 (weights pre-scaled by 16 = 2^4, exactly undone
    in the final dot weights).  Measured on CPU this gives ~2e-3 final
    rel err (the tolerance is 2e-2); the col path stays bf16 because
    quantizing it dominates the error budget.
  * L1 uses MatmulPerfMode.DoubleRow: 256-deep contraction packs FOUR
    nodes per output column at 0.5 PE cycles/col.  Two band matmuls
    (tile_position (0,0) and (0,64)) fill a dense [128, 512] PSUM tile
    with 8 nodes per column = 4096 nodes per chunk.
  * relu1 runs on the Scalar (ACT) engine (PSUM -> SBUF fp8), L2 is a
    plain fp8 matmul against blockdiag(W2 x8), relu2+row-sum runs on the
    DVE with accum_out into a per-chunk stats column.
  * x is fully resident in SBUF (1.6 MB fp8): DMA pieces land in
    disjoint regions of one tile, so there are no pool-recycle guards
    and the DMA stream never stalls on compute.  Pieces are issued from
    BOTH HWDGE queues (Sync + Scalar) to hide the ~650ns per-issue cost.
  * the final reduction is done ON the PE: stats -> row sums -> a [1,1]
    fp32 dot with node_fc_W/ (256 N) (and col_W2/C), so the output DMA
    is a single [1,2] descriptor.  (The baseline's [128,1] output DMA
    generated 128 4-byte descriptors and cost ~8us of tail latency.)
"""

import ml_dtypes
import numpy as np

import concourse.bacc as bacc
import concourse.bass as bass
import concourse.mybir as mybir
import concourse.tile as tile
from concourse.bass_utils import run_bass_kernel_spmd

P = 128
N_CORES = 8
B, N, F_NODE, H = 4, 50000, 64, 16
C, F_COL = 1000, 32
NODES = (B * N) // N_CORES                 # 25000 nodes per core
CHN = 4096                                 # nodes per PSUM chunk (8/col x 512)
NQ_FULL = NODES // CHN                     # 6 full chunks
TAIL_N = NODES - NQ_FULL * CHN             # 424
TAIL_U = TAIL_N // 8                       # 53 output columns in tail chunk
NCH = NQ_FULL + 1                          # 7 chunks total
COLN = (B * C) // N_CORES                  # 500 col-feature rows per core
WS = 16.0                                  # pow2 weight scale for fp8

F8 = mybir.dt.float8e4
NPF8 = ml_dtypes.float8_e4m3               # matches mybir.dt.np(float8e4)
BF = mybir.dt.bfloat16
NPBF = ml_dtypes.bfloat16

# wpack8 [128, 5, 64] fp8: [:,0:2,:] = W1 DoubleRow blockdiag,
# [:,2:4,:] = blockdiag(W2 x8) flattened, [:,4,:] = zeros (warmup).
# wpackb [32, 16+COLN] bf16: col_W1 | colT.
# wpack32 [128, 8] fp32: wn | wc | b1s | b2s | cb1 | pad.
WB = H + COLN

PROFILE = False
CHECK_WAITS = True
LAST_EXEC_TIME_NS = None
LAST_RESULTS = None

_NC_CACHE = {}


def _build_nc(with_bias=False):
    f32 = mybir.dt.float32
    Relu = mybir.ActivationFunctionType.Relu
    DR = mybir.MatmulPerfMode.DoubleRow
    nc = bacc.Bacc("TRN2")

    x8 = nc.dram_tensor("x8", [P, 4 * NQ_FULL, 512], F8, kind="ExternalInput")
    xt8 = nc.dram_tensor("xt8", [P, 4, TAIL_U], F8, kind="ExternalInput")
    wp8 = nc.dram_tensor("wp8", [P, 5, 64], F8, kind="ExternalInput")
    wpb = nc.dram_tensor("wpb", [F_COL, WB], BF, kind="ExternalInput")
    wp32 = nc.dram_tensor("wp32", [P, 8], f32, kind="ExternalInput")
    out = nc.dram_tensor("out", [1, 2], f32, kind="ExternalOutput")

    with tile.TileContext(nc) as tc:
        with (
            tc.tile_pool(name="consts", bufs=1) as consts,
            tc.tile_pool(name="xin", bufs=1) as xin,
            tc.tile_pool(name="work", bufs=1) as work,
            tc.tile_pool(name="psum", bufs=1, space="PSUM") as psum,
        ):
            wp8sb = consts.tile([P, 5, 64], F8)
            wpbsb = consts.tile([F_COL, WB], BF)
            wp32sb = consts.tile([P, 8], f32)
            stats = consts.tile([P, NCH + 1], f32)
            node_sb = consts.tile([P, 1], f32)
            col_sb = consts.tile([H, 1], f32)
            outsb = consts.tile([1, 2], f32)
            colscr = consts.tile([H, COLN], BF)

            xall = xin.tile([P, 4 * NQ_FULL, 512], F8)
            xtail = xin.tile([P, 4, TAIL_U], F8)
            h1r = [work.tile([P, 512], F8, tag=f"h1r{k}", name=f"h1r{k}")
                   for k in range(3)]
            scr = [work.tile([P, 512], F8, tag=f"scr{k}", name=f"scr{k}")
                   for k in range(2)]

            # DoubleRow matmul outputs must start at PSUM partition 0, so
            # the two 4-node bands land in separate half-used banks.
            ps1a = [psum.tile([64, 512], f32, tag=f"ps1a{k}", name=f"ps1a{k}")
                    for k in range(2)]
            ps1b = [psum.tile([64, 512], f32, tag=f"ps1b{k}", name=f"ps1b{k}")
                    for k in range(2)]
            ps2 = [psum.tile([P, 512], f32, tag=f"ps2_{k}", name=f"ps2_{k}")
                   for k in range(3)]
            pscm = psum.tile([H, 512], f32, tag="pscm")
            psc = pscm[:, 0:COLN]
            psd = pscm[0:1, COLN:COLN + 1]
            psdot = pscm[0:1, COLN + 1:COLN + 3]

            wl1 = wp8sb[:, 0:2, :]                  # [128, 2, 64] DoubleRow
            wl2 = wp8sb[:, 2:4, :]                  # [128, 128] flattened
            zc = wp8sb[:, 4:5, 0:1]                 # [128, 1] zeros
            cw1 = wpbsb[:, 0:H]
            colT = wpbsb[:, H:H + COLN]
            wn = wp32sb[:, 0:1]
            wc = wp32sb[:H, 1:2]

            # stats zeroed on GpSimd: no cross-lane data deps, runs during
            # the DMA window.
            nc.gpsimd.memset(stats[:], 0.0)

            # DMA issues.  Two HWDGE queues: Scalar gets chunk 0 first so
            # compute starts ASAP; Sync streams the weights pack + the rest.
            nc.scalar.dma_start(xall[:, 0:4, :], x8[:, 0:4, :])
            nc.scalar.dma_start(wp32sb[:], wp32[:])
            nc.scalar.dma_start(wpbsb[:], wpb[:])
            nc.sync.dma_start(wp8sb[:], wp8[:])
            for q in range(1, NQ_FULL):
                nc.sync.dma_start(xall[:, 4 * q:4 * q + 4, :],
                                  x8[:, 4 * q:4 * q + 4, :])
            nc.sync.dma_start(xtail[:], xt8[:])

            # Warmup matmul: subsumes the wp8 DMA wait into PE program
            # order with a single semaphore wait; copy keeps it live.
            nc.tensor.matmul(psd[0:1, 0:1], zc, zc, start=True, stop=True)
            nc.vector.tensor_copy(stats[0:1, NCH:NCH + 1], psd[0:1, 0:1])

            b1s = wp32sb[:, 2:3]
            b2s = wp32sb[:, 3:4]
            cb1 = wp32sb[:H, 4:5]

            for s in range(NCH):
                u = 512 if s < NQ_FULL else TAIL_U
                if s < NQ_FULL:
                    xa = xall[:, 4 * s:4 * s + 2, :]
                    xb = xall[:, 4 * s + 2:4 * s + 4, :]
                else:
                    xa = xtail[:, 0:2, :]
                    xb = xtail[:, 2:4, :]
                pa = ps1a[s % 2]
                pb = ps1b[s % 2]
                nc.tensor.matmul(pa[:, :u], wl1, xa, start=True, stop=True,
                                 perf_mode=DR, tile_position=(0, 0))
                nc.tensor.matmul(pb[:, :u], wl1, xb, start=True, stop=True,
                                 perf_mode=DR, tile_position=(0, 0))
                h = h1r[s % 3]
                # relu1 split across the two PSUM-capable engines: band A
                # on ACT, band B on DVE; both write halves of one h1r tile.
                nc.scalar.activation(h[0:64, :u], pa[:, :u], Relu,
                                     bias=b1s[0:64] if with_bias else 0.0)
                if with_bias:
                    # b1s is 16-periodic, so rows 0:64 serve band B as well
                    nc.vector.tensor_scalar(
                        h[64:128, :u], pb[:, :u], b1s[0:64], 0.0,
                        mybir.AluOpType.add, mybir.AluOpType.max)
                else:
                    nc.vector.tensor_scalar_max(h[64:128, :u], pb[:, :u], 0.0)
                p2 = ps2[s % 3]
                nc.tensor.matmul(p2[:, :u], wl2, h[:, :u], start=True, stop=True)
                # relu2 + row-sum accumulate: alternate engines by parity
                # to balance ACT vs DVE load.
                if with_bias:
                    if s % 2 == 0:
                        nc.vector.tensor_scalar(
                            scr[s % 2][:, :u], p2[:, :u], b2s, 0.0,
                            mybir.AluOpType.add, mybir.AluOpType.max,
                            accum_out=stats[:, s:s + 1])
                    else:
                        nc.scalar.activation(
                            scr[s % 2][:, :u], p2[:, :u], Relu, bias=b2s,
                            accum_out=stats[:, s:s + 1])
                else:
                    if s % 2 == 0:
                        nc.vector.tensor_scalar(
                            scr[s % 2][:, :u], p2[:, :u], 0.0, 0.0,
                            mybir.AluOpType.max, mybir.AluOpType.add,
                            accum_out=stats[:, s:s + 1])
                    else:
                        nc.scalar.activation(
                            scr[s % 2][:, :u], p2[:, :u], Relu,
                            accum_out=stats[:, s:s + 1])
                if s == 2:
                    # col path (bf16, tiny): fits in PE/ACT slack mid-stream
                    nc.tensor.matmul(psc[:, :], cw1, colT, start=True,
                                     stop=True)
                    nc.scalar.activation(colscr[:], psc[:], Relu,
                                         bias=cb1 if with_bias else 0.0,
                                         accum_out=col_sb[:])

            # Final reductions on-chip: row sums, then PE dot products so
            # the output is a single tiny [1,2] DMA (one descriptor).
            nc.vector.tensor_reduce(node_sb[:], stats[:],
                                    axis=mybir.AxisListType.X,
                                    op=mybir.AluOpType.add)
            nc.tensor.matmul(psdot[0:1, 0:1], node_sb[:], wn,
                             start=True, stop=True)
            nc.tensor.matmul(psdot[0:1, 1:2], col_sb[:], wc,
                             start=True, stop=True)
            nc.scalar.copy(outsb[:], psdot[:])
            nc.sync.dma_start(out[:], outsb[:])

    nc.finalize()

    if CHECK_WAITS:
        for blk in nc.m.functions[0].blocks:
            for inst in blk.instructions:
                si = inst.sync_info
                nwait = len(si.on_wait) if si and si.on_wait else 0
                limit = 2 if type(inst).__name__ in (
                    "InstEventSemaphore", "InstDrain", "InstDMACopy") else 1
                assert nwait <= limit, (
                    inst.name, type(inst).__name__,
                    [w.ant_name for w in si.on_wait])
    return nc


def _get_nc(with_bias=False):
    key = ("nc", with_bias)
    if key not in _NC_CACHE:
        _NC_CACHE[key] = _build_nc(with_bias)
    return _NC_CACHE[key]


def _pack_x(node_features):
    """[B,N,64] -> per-core [128, 24, 512] + [128, 4, 53] fp8 arrays.

    Node n (within a core) lives at chunk q = n // 4096, slot
    s = (n % 4096) // 512 = band*4 + t*2 + half, column u = n % 512:
    SBUF block index 4q + band*2 + t, partitions half*64 + f.
    """
    x = np.ascontiguousarray(node_features, np.float32).reshape(
        N_CORES, NODES, F_NODE)
    main = x[:, :NQ_FULL * CHN].reshape(N_CORES, NQ_FULL, 2, 2, 2, 512, F_NODE)
    #                  core, q, band, t, half, u, f -> core, half, f, q, band, t, u
    main = main.transpose(0, 4, 6, 1, 2, 3, 5).reshape(
        N_CORES, P, 4 * NQ_FULL, 512)
    tail = x[:, NQ_FULL * CHN:].reshape(N_CORES, 2, 2, 2, TAIL_U, F_NODE)
    tail = tail.transpose(0, 3, 5, 1, 2, 4).reshape(N_CORES, P, 4, TAIL_U)
    return main.astype(NPF8), tail.astype(NPF8)


def _prep_in_maps(node_features, col_features, W1, b1, W2, b2,
                  node_fc_W, col_W1, col_b1, col_W2):
    W1s = (np.asarray(W1, np.float32) * WS)
    W2s = (np.asarray(W2, np.float32) * WS)

    wl1 = np.zeros((P, 2, 64), np.float32)
    wl1[0:64, 0, 0:H] = W1s
    wl1[64:128, 0, H:2 * H] = W1s
    wl1[0:64, 1, 2 * H:3 * H] = W1s
    wl1[64:128, 1, 3 * H:4 * H] = W1s
    wl2 = np.zeros((P, P), np.float32)
    for g in range(P // H):
        wl2[H * g:H * g + H, H * g:H * g + H] = W2s
    wp8 = np.zeros((P, 5, 64), np.float32)
    wp8[:, 0:2, :] = wl1
    wp8[:, 2:4, :] = wl2.reshape(P, 2, 64)
    wp8 = wp8.astype(NPF8)

    colf = np.ascontiguousarray(col_features, np.float32).reshape(B * C, F_COL)
    wpb_base = np.zeros((F_COL, WB), np.float32)
    wpb_base[:, 0:H] = np.asarray(col_W1, np.float32)

    wp32 = np.zeros((P, 8), np.float32)
    wp32[:, 0] = np.tile(np.asarray(node_fc_W, np.float32)[:, 0], P // H) \
        / (WS * WS * np.float32(N))
    wp32[:H, 1] = np.asarray(col_W2, np.float32)[:, 0] / np.float32(C)
    wp32[:, 2] = WS * np.tile(np.asarray(b1, np.float32), P // H)
    wp32[:, 3] = WS * WS * np.tile(np.asarray(b2, np.float32), P // H)
    wp32[:H, 4] = np.asarray(col_b1, np.float32)

    xmain, xtail = _pack_x(node_features)

    in_maps = []
    for c in range(N_CORES):
        wpb = wpb_base.copy()
        wpb[:, H:] = colf[c * COLN:(c + 1) * COLN].T
        in_maps.append({
            "x8": xmain[c],
            "xt8": xtail[c],
            "wp8": wp8,
            "wpb": wpb.astype(NPBF),
            "wp32": wp32,
        })
    return in_maps


def kernel(node_features, col_features, edge_index, W1, b1, W2, b2,
           node_fc_W, node_fc_b, col_W1, col_b1, col_W2, col_b2,
           fc_W, fc_b, out_W, out_b):
    global LAST_EXEC_TIME_NS, LAST_RESULTS
    # edge_index provably does not affect the output (see module docstring).
    in_maps = _prep_in_maps(node_features, col_features, W1, b1, W2, b2,
                            node_fc_W, col_W1, col_b1, col_W2)
    with_bias = bool(np.any(np.asarray(b1)) or np.any(np.asarray(b2))
                     or np.any(np.asarray(col_b1)))
    nc = _get_nc(with_bias)
    res = run_bass_kernel_spmd(nc, in_maps, core_ids=list(range(N_CORES)),
                               trace=PROFILE)
    LAST_EXEC_TIME_NS = res.exec_time_ns
    LAST_RESULTS = res
    outs = res.results

    node_avg = np.zeros((B, 1), np.float32)
    col_avg = np.zeros((B, 1), np.float32)
    nfb = np.asarray(node_fc_b, np.float32)[0]
    cb2 = np.asarray(col_b2, np.float32)[0]
    for b in range(B):
        o0 = outs[2 * b]["out"].reshape(2)
        o1 = outs[2 * b + 1]["out"].reshape(2)
        node_avg[b, 0] = o0[0] + o1[0] + nfb
        col_avg[b, 0] = o0[1] + o1[1] + cb2

    combined = np.concatenate([node_avg, col_avg], axis=1)      # [B, 2]
    z = np.maximum(combined @ np.asarray(fc_W, np.float32) +
                   np.asarray(fc_b, np.float32), 0.0)
    out = z @ np.asarray(out_W, np.float32) + np.asarray(out_b, np.float32)
    return out.astype(np.float32)


# revision 9
# speedup vs baseline: 1.3390x; 1.0441x over previous
"""Trainium2 Bass kernel for nn_CriticNetwork (gnn_message_passing).

Math (verified vs reference): the reference broadcasts edge_index to
(B, 2, E) and reshapes to (2, B*E); row-major interleaving makes src and
dst elementwise equal, so every edge is a self-edge and both GCNConv
layers collapse to plain linear layers (deg*x/deg = x):

    x = relu(x @ W1); x = relu(x @ W2)        (biases are zeros)
    node_avg[b] = mean_n(x[b,n] @ node_fc_W) + node_fc_b
    col path is a tiny 2-layer MLP; final head is a [4,2] MLP.

Device strategy (per core, 25000 nodes = half a batch):
  * node path in fp8e4m3 end to end (x, W1*16, W2*16, hidden acts);
    ~2e-3 final rel err vs the 2e-2 gate.  The tiny col path stays bf16
    (quantizing it dominates the error budget).
  * L1 uses MatmulPerfMode.DoubleRow: 256-deep contraction packs FOUR
    nodes per output column.  Outputs must start at PSUM partition 0,
    so the two 4-node bands go to separate [64,512] banks; relu1 band A
    runs on ACT and band B on DVE, writing halves of one dense
    [128,512] h1r tile; L2 is one plain fp8 matmul per chunk; relu2 +
    row-sum accum alternates ACT/DVE by chunk parity.
  * ALL inputs ship in ONE fp8 DRAM tensor (weights fp8 | bf16 block as
    raw bytes via AP.bitcast | x blocks), moved by 5 column-range DMAs
    round-robined over the two HWDGE queues (Sync + Scalar) so the
    hardware descriptor generators work in parallel; x stays fully
    resident in SBUF (no pool recycling, DMA never stalls on compute).
  * the final reduction is done ON the PE: stats row-sums (bf16) dot
    node_fc_W/(256N) and col_W2/C, so the output DMA is a single [1,2]
    fp32 descriptor.  (A [128,1] output DMA costs ~128 descriptors and
    microseconds of tail latency.)
"""

import ml_dtypes
import numpy as np

import concourse.bacc as bacc
import concourse.bass as bass
import concourse.mybir as mybir
import concourse.tile as tile
from concourse.bass_utils import run_bass_kernel_spmd

P = 128
N_CORES = 8
B, N, F_NODE, H = 4, 50000, 64, 16
C, F_COL = 1000, 32
NODES = (B * N) // N_CORES                 # 25000 nodes per core
CHN = 4096                                 # nodes per PSUM chunk (8/col x 512)
NQ_FULL = NODES // CHN                     # 6 full chunks
TAIL_N = NODES - NQ_FULL * CHN             # 424
TAIL_U = TAIL_N // 8                       # 53 output columns in tail chunk
NCH = NQ_FULL + 1                          # 7 chunks total
COLN = (B * C) // N_CORES                  # 500 col-feature rows per core
WS = 16.0                                  # pow2 weight scale for fp8

F8 = mybir.dt.float8e4
NPF8 = ml_dtypes.float8_e4m3               # matches mybir.dt.np(float8e4)
BF = mybir.dt.bfloat16
NPBF = ml_dtypes.bfloat16

# Single packed input tensor xw8 [128, TOT] fp8 (per core), columns:
#   [0:128)        wl1: W1 DoubleRow blockdiag ([128,2,64] view)
#   [128:256)      wl2: blockdiag(W2 x8) flattened
#   [256:257)      zeros column (warmup operand)
#   [320:...)      bf16 block as raw bytes (bitcast view [128, WBX]):
#                  bf16 cols 0:16 col_W1 (rows 0-31), 16:516 colT
#                  (rows 0-31), 516 wn, 517 wc, 518 b1s, 519 b2s, 520 cb1
#   [X0:X0+12288)  x main blocks [24, 512]
#   [XT:XT+212)    x tail blocks [4, 53]
WBX = H + COLN + 5                          # 521 bf16 columns
BF_OFF = 320
X0 = BF_OFF + 2 * WBX                       # 1362 -> pad to 1364
X0 = (X0 + 3) & ~3                          # 1364
XT = X0 + NQ_FULL * 2048                    # 13652
TOT = XT + 4 * TAIL_U                       # 13864

PROFILE = False
CHECK_WAITS = True
LAST_EXEC_TIME_NS = None
LAST_RESULTS = None

_NC_CACHE = {}


def _build_nc(with_bias=False):
    f32 = mybir.dt.float32
    Relu = mybir.ActivationFunctionType.Relu
    DR = mybir.MatmulPerfMode.DoubleRow
    nc = bacc.Bacc("TRN2")

    xw8 = nc.dram_tensor("xw8", [P, TOT], F8, kind="ExternalInput")
    out = nc.dram_tensor("out", [1, 2], f32, kind="ExternalOutput")

    with tile.TileContext(nc) as tc:
        with (
            tc.tile_pool(name="consts", bufs=1) as consts,
            tc.tile_pool(name="xin", bufs=1) as xin,
            tc.tile_pool(name="work", bufs=1) as work,
            tc.tile_pool(name="psum", bufs=1, space="PSUM") as psum,
        ):
            xf = xin.tile([P, TOT], F8)
            stats = consts.tile([P, NCH + 1], f32)
            node_sb = consts.tile([P, 1], BF)
            col_sb = consts.tile([H, 1], f32)
            col_sbb = consts.tile([H, 1], BF)
            outsb = consts.tile([1, 2], f32)
            colscr = consts.tile([H, COLN], BF)
            h1r = [work.tile([P, 512], F8, tag=f"h1r{k}", name=f"h1r{k}")
                   for k in range(3)]
            scr = [work.tile([P, 512], F8, tag=f"scr{k}", name=f"scr{k}")
                   for k in range(2)]

            # DoubleRow matmul outputs must start at PSUM partition 0, so
            # the two 4-node bands land in separate half-used banks.
            ps1a = [psum.tile([64, 512], f32, tag=f"ps1a{k}", name=f"ps1a{k}")
                    for k in range(2)]
            ps1b = [psum.tile([64, 512], f32, tag=f"ps1b{k}", name=f"ps1b{k}")
                    for k in range(2)]
            ps2 = [psum.tile([P, 512], f32, tag=f"ps2_{k}", name=f"ps2_{k}")
                   for k in range(3)]
            pscm = psum.tile([H, 512], f32, tag="pscm")
            psc = pscm[:, 0:COLN]
            psd = pscm[0:1, COLN:COLN + 1]
            psdot = pscm[0:1, COLN + 1:COLN + 3]

            wl1 = xf[:, 0:128].rearrange("p (t m) -> p t m", t=2)
            wl2 = xf[:, 128:256]
            zc = xf[:, 256:257]
            wbx = xf[:, BF_OFF:BF_OFF + 2 * WBX].bitcast(BF)
            cw1 = wbx[0:F_COL, 0:H]
            colT = wbx[0:F_COL, H:H + COLN]
            wn = wbx[:, H + COLN:H + COLN + 1]
            wc = wbx[0:H, H + COLN + 1:H + COLN + 2]
            b1s = wbx[:, H + COLN + 2:H + COLN + 3]
            b2s = wbx[:, H + COLN + 3:H + COLN + 4]
            cb1 = wbx[0:H, H + COLN + 4:H + COLN + 5]

            # stats zeroed on GpSimd: no data deps, runs during DMA window.
            nc.gpsimd.memset(stats[:], 0.0)

            # 5 column-range DMA pieces round-robined over the two HWDGE
            # queues (parallel descriptor generation): chunk 0 first.
            nc.sync.dma_start(xf[:, X0:X0 + 2048], xw8[:, X0:X0 + 2048])
            nc.scalar.dma_start(xf[:, 0:X0], xw8[:, 0:X0])
            nc.sync.dma_start(xf[:, X0 + 2048:X0 + 6144],
                              xw8[:, X0 + 2048:X0 + 6144])
            nc.scalar.dma_start(xf[:, X0 + 6144:X0 + 10240],
                                xw8[:, X0 + 6144:X0 + 10240])
            nc.sync.dma_start(xf[:, X0 + 10240:TOT], xw8[:, X0 + 10240:TOT])

            # Warmup matmul: subsumes the consts-piece DMA wait into PE
            # program order with a single semaphore wait.
            nc.tensor.matmul(psd, zc, zc, start=True, stop=True)
            nc.vector.tensor_copy(stats[0:1, NCH:NCH + 1], psd)

            for s in range(NCH):
                u = 512 if s < NQ_FULL else TAIL_U
                base = X0 + 2048 * s
                xa = xf[:, base:base + 2 * u].rearrange("p (t u) -> p t u", t=2)
                xb = xf[:, base + 2 * u:base + 4 * u].rearrange(
                    "p (t u) -> p t u", t=2)
                pa = ps1a[s % 2]
                pb = ps1b[s % 2]
                nc.tensor.matmul(pa[:, :u], wl1, xa, start=True, stop=True,
                                 perf_mode=DR, tile_position=(0, 0))
                nc.tensor.matmul(pb[:, :u], wl1, xb, start=True, stop=True,
                                 perf_mode=DR, tile_position=(0, 0))
                h = h1r[s % 3]
                # relu1 split across the two PSUM-capable engines: band A
                # on ACT, band B on DVE; both write halves of one h1r tile.
                nc.scalar.activation(h[0:64, :u], pa[:, :u], Relu,
                                     bias=b1s[0:64] if with_bias else 0.0)
                if with_bias:
                    # b1s is 16-periodic, so rows 0:64 serve band B as well
                    nc.vector.tensor_scalar(
                        h[64:128, :u], pb[:, :u], b1s[0:64], 0.0,
                        mybir.AluOpType.add, mybir.AluOpType.max)
                else:
                    nc.vector.tensor_scalar_max(h[64:128, :u], pb[:, :u], 0.0)
                p2 = ps2[s % 3]
                nc.tensor.matmul(p2[:, :u], wl2, h[:, :u], start=True, stop=True)
                # relu2 + row-sum accumulate: alternate engines by parity
                # to balance ACT vs DVE load.
                if with_bias:
                    if s % 2 == 0:
                        nc.vector.tensor_scalar(
                            scr[s % 2][:, :u], p2[:, :u], b2s, 0.0,
                            mybir.AluOpType.add, mybir.AluOpType.max,
                            accum_out=stats[:, s:s + 1])
                    else:
                        nc.scalar.activation(
                            scr[s % 2][:, :u], p2[:, :u], Relu, bias=b2s,
                            accum_out=stats[:, s:s + 1])
                else:
                    if s % 2 == 0:
                        nc.vector.tensor_scalar(
                            scr[s % 2][:, :u], p2[:, :u], 0.0, 0.0,
                            mybir.AluOpType.max, mybir.AluOpType.add,
                            accum_out=stats[:, s:s + 1])
                    else:
                        nc.scalar.activation(
                            scr[s % 2][:, :u], p2[:, :u], Relu,
                            accum_out=stats[:, s:s + 1])
                if s == 2:
                    # col path (bf16, tiny): fits in PE/ACT slack mid-stream
                    nc.tensor.matmul(psc[:, :], cw1, colT, start=True,
                                     stop=True)
                    nc.scalar.activation(colscr[:], psc[:], Relu,
                                         bias=cb1 if with_bias else 0.0,
                                         accum_out=col_sb[:])
                    nc.vector.tensor_copy(col_sbb[:], col_sb[:])

            # Final reductions on-chip: bf16 row sums, then PE dot products
            # so the output is a single tiny [1,2] DMA (one descriptor).
            with nc.allow_low_precision(
                    reason="bf16 row-sums feed a bf16 PE dot; 8 fp32 "
                           "terms/partition, ~0.4% quantization on a "
                           "2e-2 budget"):
                nc.vector.tensor_reduce(node_sb[:], stats[:],
                                        axis=mybir.AxisListType.X,
                                        op=mybir.AluOpType.add)
            nc.tensor.matmul(psdot[0:1, 0:1], node_sb[:], wn,
                             start=True, stop=True)
            nc.tensor.matmul(psdot[0:1, 1:2], col_sbb[:], wc,
                             start=True, stop=True)
            nc.scalar.copy(outsb[:], psdot[:])
            nc.sync.dma_start(out[:], outsb[:])

    nc.finalize()

    if CHECK_WAITS:
        for blk in nc.m.functions[0].blocks:
            for inst in blk.instructions:
                si = inst.sync_info
                nwait = len(si.on_wait) if si and si.on_wait else 0
                limit = 2 if type(inst).__name__ in (
                    "InstEventSemaphore", "InstDrain", "InstDMACopy") else 1
                assert nwait <= limit, (
                    inst.name, type(inst).__name__,
                    [w.ant_name for w in si.on_wait])
    return nc


def _get_nc(with_bias=False):
    key = ("nc", with_bias)
    if key not in _NC_CACHE:
        _NC_CACHE[key] = _build_nc(with_bias)
    return _NC_CACHE[key]


def _pack_x(node_features):
    """[B,N,64] -> per-core [128, 12288] + [128, 212] fp8 x payloads.

    Node n (within a core) lives at chunk q = n // 4096, slot
    s = (n % 4096) // 512 = band*4 + t*2 + half, column u = n % 512:
    columns q*2048 + band*1024 + t*512 + u, partitions half*64 + f.
    """
    x = np.ascontiguousarray(node_features, np.float32).reshape(
        N_CORES, NODES, F_NODE)
    main = x[:, :NQ_FULL * CHN].reshape(N_CORES, NQ_FULL, 2, 2, 2, 512, F_NODE)
    #                  core, q, band, t, half, u, f -> core, half, f, q, band, t, u
    main = main.transpose(0, 4, 6, 1, 2, 3, 5).reshape(
        N_CORES, P, NQ_FULL * 2048)
    tail = x[:, NQ_FULL * CHN:].reshape(N_CORES, 2, 2, 2, TAIL_U, F_NODE)
    tail = tail.transpose(0, 3, 5, 1, 2, 4).reshape(N_CORES, P, 4 * TAIL_U)
    return main.astype(NPF8), tail.astype(NPF8)


def _prep_in_maps(node_features, col_features, W1, b1, W2, b2,
                  node_fc_W, col_W1, col_b1, col_W2):
    W1s = np.asarray(W1, np.float32) * WS
    W2s = np.asarray(W2, np.float32) * WS

    wl1 = np.zeros((P, 2, 64), np.float32)
    wl1[0:64, 0, 0:H] = W1s
    wl1[64:128, 0, H:2 * H] = W1s
    wl1[0:64, 1, 2 * H:3 * H] = W1s
    wl1[64:128, 1, 3 * H:4 * H] = W1s
    wl2 = np.zeros((P, P), np.float32)
    for g in range(P // H):
        wl2[H * g:H * g + H, H * g:H * g + H] = W2s

    wbx_base = np.zeros((P, WBX), np.float32)
    wbx_base[0:F_COL, 0:H] = np.asarray(col_W1, np.float32)
    wbx_base[:, H + COLN] = np.tile(np.asarray(node_fc_W, np.float32)[:, 0],
                                    P // H) / (WS * WS * np.float32(N))
    wbx_base[0:H, H + COLN + 1] = np.asarray(col_W2, np.float32)[:, 0] \
        / np.float32(C)
    wbx_base[:, H + COLN + 2] = WS * np.tile(np.asarray(b1, np.float32),
                                             P // H)
    wbx_base[:, H + COLN + 3] = WS * WS * np.tile(np.asarray(b2, np.float32),
                                                  P // H)
    wbx_base[0:H, H + COLN + 4] = np.asarray(col_b1, np.float32)

    colf = np.ascontiguousarray(col_features, np.float32).reshape(B * C, F_COL)
    xmain, xtail = _pack_x(node_features)

    base = np.zeros((P, TOT), NPF8)
    bb = base.view(np.uint8)
    bb[:, 0:128] = wl1.reshape(P, 128).astype(NPF8).view(np.uint8)
    bb[:, 128:256] = wl2.astype(NPF8).view(np.uint8)

    in_maps = []
    for c in range(N_CORES):
        arr = base.copy()
        ab = arr.view(np.uint8)
        wbx = wbx_base.copy()
        wbx[0:F_COL, H:H + COLN] = colf[c * COLN:(c + 1) * COLN].T
        ab[:, BF_OFF:BF_OFF + 2 * WBX] = \
            wbx.astype(NPBF).view(np.uint8).reshape(P, 2 * WBX)
        ab[:, X0:XT] = xmain[c].view(np.uint8)
        ab[:, XT:TOT] = xtail[c].view(np.uint8)
        in_maps.append({"xw8": arr})
    return in_maps


def kernel(node_features, col_features, edge_index, W1, b1, W2, b2,
           node_fc_W, node_fc_b, col_W1, col_b1, col_W2, col_b2,
           fc_W, fc_b, out_W, out_b):
    global LAST_EXEC_TIME_NS, LAST_RESULTS
    # edge_index provably does not affect the output (see module docstring).
    in_maps = _prep_in_maps(node_features, col_features, W1, b1, W2, b2,
                            node_fc_W, col_W1, col_b1, col_W2)
    with_bias = bool(np.any(np.asarray(b1)) or np.any(np.asarray(b2))
                     or np.any(np.asarray(col_b1)))
    nc = _get_nc(with_bias)
    res = run_bass_kernel_spmd(nc, in_maps, core_ids=list(range(N_CORES)),
                               trace=PROFILE)
    LAST_EXEC_TIME_NS = res.exec_time_ns
    LAST_RESULTS = res
    outs = res.results

    node_avg = np.zeros((B, 1), np.float32)
    col_avg = np.zeros((B, 1), np.float32)
    nfb = np.asarray(node_fc_b, np.float32)[0]
    cb2 = np.asarray(col_b2, np.float32)[0]
    for b in range(B):
        o0 = outs[2 * b]["out"].reshape(2)
        o1 = outs[2 * b + 1]["out"].reshape(2)
        node_avg[b, 0] = o0[0] + o1[0] + nfb
        col_avg[b, 0] = o0[1] + o1[1] + cb2

    combined = np.concatenate([node_avg, col_avg], axis=1)      # [B, 2]
    z = np.maximum(combined @ np.asarray(fc_W, np.float32) +
                   np.asarray(fc_b, np.float32), 0.0)
    out = z @ np.asarray(out_W, np.float32) + np.asarray(out_b, np.float32)
    return out.astype(np.float32)
